# revision 3
# baseline (speedup 1.0000x reference)
"""ALiBi-2D bias-add kernel for 8 Trainium2 NeuronCores.

out[b,h,i,j] = attn_scores[b,h,i,j] - slopes[h] * dist(coords[b,i], coords[b,j])

Sharding: the 32 (b,h) slices are split 4-per-core across 8 cores (all four
heads on a core share the same batch b, so the pairwise-distance tile is
computed once per row-tile and reused for all 4 heads).
"""

import numpy as np

import concourse.bacc as bacc
import concourse.mybir as mybir
from concourse.bass_utils import run_bass_kernel_spmd
from concourse.tile import TileContext

B, H, T = 2, 16, 2048
P = 128
NT = T // P  # row tiles per core
NCORES = 8
CPB = NCORES // B  # cores per batch entry (4)
HPC = H // CPB  # heads per core (4)

F32 = mybir.dt.float32


def _build_nc():
    nc = bacc.Bacc(
        "TRN2", target_bir_lowering=False, debug=False, num_devices=NCORES
    )
    AF = mybir.ActivationFunctionType
    OP = mybir.AluOpType

    scores = nc.dram_tensor("scores", [HPC, T, T], F32, kind="ExternalInput")
    xb = nc.dram_tensor("xb", [P, T], F32, kind="ExternalInput")
    yb = nc.dram_tensor("yb", [P, T], F32, kind="ExternalInput")
    negx = nc.dram_tensor("negx", [P, NT], F32, kind="ExternalInput")
    negy = nc.dram_tensor("negy", [P, NT], F32, kind="ExternalInput")
    nslope = nc.dram_tensor("nslope", [P, HPC], F32, kind="ExternalInput")
    out = nc.dram_tensor("out", [HPC, T, T], F32, kind="ExternalOutput")

    with TileContext(nc) as tc:
        with (
            tc.tile_pool(name="const", bufs=1) as cpool,
            tc.tile_pool(name="dxy", bufs=2) as dxy_pool,
            tc.tile_pool(name="dist", bufs=2) as dist_pool,
            tc.tile_pool(name="sin", bufs=6) as sin_pool,
            tc.tile_pool(name="sout", bufs=6) as sout_pool,
        ):
            xb_t = cpool.tile([P, T], F32)
            yb_t = cpool.tile([P, T], F32)
            negx_t = cpool.tile([P, NT], F32)
            negy_t = cpool.tile([P, NT], F32)
            nslope_t = cpool.tile([P, HPC], F32)
            nc.sync.dma_start(out=xb_t[:], in_=xb[:])
            nc.sync.dma_start(out=yb_t[:], in_=yb[:])
            nc.sync.dma_start(out=negx_t[:], in_=negx[:])
            nc.sync.dma_start(out=negy_t[:], in_=negy[:])
            nc.sync.dma_start(out=nslope_t[:], in_=nslope[:])

            for t in range(NT):
                r0 = t * P
                dx2 = dxy_pool.tile([P, T], F32, tag="dx2")
                dy2 = dxy_pool.tile([P, T], F32, tag="dy2")
                # (xj - xi)^2 : Square(xb * 1 + (-xi)), bias is per-partition
                nc.scalar.activation(
                    dx2[:], xb_t[:], AF.Square, bias=negx_t[:, t : t + 1]
                )
                nc.scalar.activation(
                    dy2[:], yb_t[:], AF.Square, bias=negy_t[:, t : t + 1]
                )
                nc.vector.tensor_tensor(dx2[:], dx2[:], dy2[:], OP.add)
                dist = dist_pool.tile([P, T], F32)
                nc.scalar.activation(dist[:], dx2[:], AF.Sqrt)
                for h in range(HPC):
                    s = sin_pool.tile([P, T], F32)
                    nc.sync.dma_start(out=s[:], in_=scores[h, r0 : r0 + P, :])
                    o = sout_pool.tile([P, T], F32)
                    # out = (dist * -slope_h) + scores
                    nc.vector.scalar_tensor_tensor(
                        out=o[:],
                        in0=dist[:],
                        scalar=nslope_t[:, h : h + 1],
                        in1=s[:],
                        op0=OP.mult,
                        op1=OP.add,
                    )
                    nc.sync.dma_start(out=out[h, r0 : r0 + P, :], in_=o[:])
    nc.compile()
    return nc


def _shard_inputs(attn_scores, coords_xy, slopes):
    coords = coords_xy.astype(np.float32)
    slopes = np.asarray(slopes, dtype=np.float32)
    in_maps = []
    for c in range(NCORES):
        b = c // CPB
        h0 = (c % CPB) * HPC
        x = coords[b, :, 0]
        y = coords[b, :, 1]
        in_maps.append(
            {
                "scores": np.ascontiguousarray(attn_scores[b, h0 : h0 + HPC]),
                "xb": np.ascontiguousarray(np.broadcast_to(x[None, :], (P, T))),
                "yb": np.ascontiguousarray(np.broadcast_to(y[None, :], (P, T))),
                "negx": np.ascontiguousarray(-x.reshape(NT, P).T),
                "negy": np.ascontiguousarray(-y.reshape(NT, P).T),
                "nslope": np.ascontiguousarray(
                    np.broadcast_to(-slopes[h0 : h0 + HPC][None, :], (P, HPC))
                ),
            }
        )
    return in_maps


def _run(attn_scores, coords_xy, slopes, trace=False):
    attn_scores = np.asarray(attn_scores, dtype=np.float32)
    coords_xy = np.asarray(coords_xy)
    nc = _build_nc()
    in_maps = _shard_inputs(attn_scores, coords_xy, slopes)
    res = run_bass_kernel_spmd(nc, in_maps, core_ids=list(range(NCORES)), trace=trace)
    full = np.empty((B, H, T, T), dtype=np.float32)
    for c in range(NCORES):
        b = c // CPB
        h0 = (c % CPB) * HPC
        full[b, h0 : h0 + HPC] = res.results[c]["out"]
    return full, res


def kernel(attn_scores, coords_xy, slopes):
    full, _ = _run(attn_scores, coords_xy, slopes, trace=False)
    return full


# revision 5
# speedup vs baseline: 1.0467x; 1.0467x over previous
"""ALiBi-2D bias-add kernel for 8 Trainium2 NeuronCores.

out[b,h,i,j] = attn_scores[b,h,i,j] - slopes[h] * dist(coords[b,i], coords[b,j])

Sharding: the 32 (b,h) slices are split 4-per-core across 8 cores (all four
heads on a core share the same batch b, so the pairwise-distance tile is
computed once per row-tile and reused for all 4 heads).
"""

import numpy as np

import concourse.bacc as bacc
import concourse.mybir as mybir
from concourse.bass_utils import run_bass_kernel_spmd
from concourse.tile import TileContext

B, H, T = 2, 16, 2048
P = 128
NT = T // P  # row tiles per core
NCORES = 8
CPB = NCORES // B  # cores per batch entry (4)
HPC = H // CPB  # heads per core (4)

F32 = mybir.dt.float32


def _build_nc():
    nc = bacc.Bacc(
        "TRN2", target_bir_lowering=False, debug=False, num_devices=NCORES
    )
    AF = mybir.ActivationFunctionType
    OP = mybir.AluOpType

    scores = nc.dram_tensor("scores", [HPC, T, T], F32, kind="ExternalInput")
    xb = nc.dram_tensor("xb", [P, T], F32, kind="ExternalInput")
    yb = nc.dram_tensor("yb", [P, T], F32, kind="ExternalInput")
    negx = nc.dram_tensor("negx", [P, NT], F32, kind="ExternalInput")
    negy = nc.dram_tensor("negy", [P, NT], F32, kind="ExternalInput")
    nslope = nc.dram_tensor("nslope", [P, HPC], F32, kind="ExternalInput")
    out = nc.dram_tensor("out", [HPC, T, T], F32, kind="ExternalOutput")

    with TileContext(nc) as tc:
        with (
            tc.tile_pool(name="const", bufs=1) as cpool,
            tc.tile_pool(name="dxy", bufs=2) as dxy_pool,
            tc.tile_pool(name="dist", bufs=2) as dist_pool,
            tc.tile_pool(name="sin", bufs=8) as sin_pool,
            tc.tile_pool(name="sout", bufs=7) as sout_pool,
        ):
            xb_t = cpool.tile([P, T], F32)
            yb_t = cpool.tile([P, T], F32)
            negx_t = cpool.tile([P, NT], F32)
            negy_t = cpool.tile([P, NT], F32)
            nslope_t = cpool.tile([P, HPC], F32)
            nc.sync.dma_start(out=xb_t[:], in_=xb[:])
            nc.sync.dma_start(out=yb_t[:], in_=yb[:])
            nc.sync.dma_start(out=negx_t[:], in_=negx[:])
            nc.sync.dma_start(out=negy_t[:], in_=negy[:])
            nc.sync.dma_start(out=nslope_t[:], in_=nslope[:])

            for t in range(NT):
                r0 = t * P
                dx2 = dxy_pool.tile([P, T], F32, tag="dx2")
                dy2 = dxy_pool.tile([P, T], F32, tag="dy2")
                # (xj - xi)^2 : Square(xb * 1 + (-xi)), bias is per-partition
                nc.scalar.activation(
                    dx2[:], xb_t[:], AF.Square, bias=negx_t[:, t : t + 1]
                )
                nc.scalar.activation(
                    dy2[:], yb_t[:], AF.Square, bias=negy_t[:, t : t + 1]
                )
                nc.vector.tensor_tensor(dx2[:], dx2[:], dy2[:], OP.add)
                dist = dist_pool.tile([P, T], F32)
                nc.scalar.activation(dist[:], dx2[:], AF.Sqrt)
                for h in range(HPC):
                    s = sin_pool.tile([P, T], F32)
                    nc.sync.dma_start(out=s[:], in_=scores[h, r0 : r0 + P, :])
                    o = sout_pool.tile([P, T], F32)
                    # out = (dist * -slope_h) + scores
                    nc.vector.scalar_tensor_tensor(
                        out=o[:],
                        in0=dist[:],
                        scalar=nslope_t[:, h : h + 1],
                        in1=s[:],
                        op0=OP.mult,
                        op1=OP.add,
                    )
                    # stores on the gpsimd SWDGE path so a store waiting on
                    # compute can't head-of-line-block loads on the sync ring
                    nc.gpsimd.dma_start(out=out[h, r0 : r0 + P, :], in_=o[:])
    nc.compile()
    return nc


def _shard_inputs(attn_scores, coords_xy, slopes):
    coords = coords_xy.astype(np.float32)
    slopes = np.asarray(slopes, dtype=np.float32)
    in_maps = []
    for c in range(NCORES):
        b = c // CPB
        h0 = (c % CPB) * HPC
        x = coords[b, :, 0]
        y = coords[b, :, 1]
        in_maps.append(
            {
                "scores": np.ascontiguousarray(attn_scores[b, h0 : h0 + HPC]),
                "xb": np.ascontiguousarray(np.broadcast_to(x[None, :], (P, T))),
                "yb": np.ascontiguousarray(np.broadcast_to(y[None, :], (P, T))),
                "negx": np.ascontiguousarray(-x.reshape(NT, P).T),
                "negy": np.ascontiguousarray(-y.reshape(NT, P).T),
                "nslope": np.ascontiguousarray(
                    np.broadcast_to(-slopes[h0 : h0 + HPC][None, :], (P, HPC))
                ),
            }
        )
    return in_maps


def _run(attn_scores, coords_xy, slopes, trace=False):
    attn_scores = np.asarray(attn_scores, dtype=np.float32)
    coords_xy = np.asarray(coords_xy)
    nc = _build_nc()
    in_maps = _shard_inputs(attn_scores, coords_xy, slopes)
    res = run_bass_kernel_spmd(nc, in_maps, core_ids=list(range(NCORES)), trace=trace)
    full = np.empty((B, H, T, T), dtype=np.float32)
    for c in range(NCORES):
        b = c // CPB
        h0 = (c % CPB) * HPC
        full[b, h0 : h0 + HPC] = res.results[c]["out"]
    return full, res


def kernel(attn_scores, coords_xy, slopes):
    full, _ = _run(attn_scores, coords_xy, slopes, trace=False)
    return full


# revision 6
# speedup vs baseline: 1.2342x; 1.1791x over previous
"""ALiBi-2D bias-add kernel for 8 Trainium2 NeuronCores.

out[b,h,i,j] = attn_scores[b,h,i,j] - slopes[h] * dist(coords[b,i], coords[b,j])

Sharding: the 32 (b,h) slices are split 4-per-core across 8 cores (all four
heads on a core share the same batch b, so the pairwise-distance tile is
computed once per row-tile and reused for all 4 heads).
"""

import numpy as np

import concourse.bacc as bacc
import concourse.mybir as mybir
from concourse.bass_utils import run_bass_kernel_spmd
from concourse.tile import TileContext

B, H, T = 2, 16, 2048
P = 128
NT = T // P  # row tiles per core
NCORES = 8
CPB = NCORES // B  # cores per batch entry (4)
HPC = H // CPB  # heads per core (4)

F32 = mybir.dt.float32


def _build_nc():
    nc = bacc.Bacc(
        "TRN2", target_bir_lowering=False, debug=False, num_devices=NCORES
    )
    AF = mybir.ActivationFunctionType
    OP = mybir.AluOpType

    scores = nc.dram_tensor("scores", [HPC, T, T], F32, kind="ExternalInput")
    xb = nc.dram_tensor("xb", [P, T], F32, kind="ExternalInput")
    yb = nc.dram_tensor("yb", [P, T], F32, kind="ExternalInput")
    negx = nc.dram_tensor("negx", [P, NT], F32, kind="ExternalInput")
    negy = nc.dram_tensor("negy", [P, NT], F32, kind="ExternalInput")
    nslope = nc.dram_tensor("nslope", [P, HPC], F32, kind="ExternalInput")
    out = nc.dram_tensor("out", [HPC, T, T], F32, kind="ExternalOutput")

    with TileContext(nc) as tc:
        with (
            tc.tile_pool(name="const", bufs=1) as cpool,
            tc.tile_pool(name="dxy", bufs=2) as dxy_pool,
            tc.tile_pool(name="dist", bufs=2) as dist_pool,
            tc.tile_pool(name="sin", bufs=8) as sin_pool,
            tc.tile_pool(name="sout", bufs=7) as sout_pool,
        ):
            xb_t = cpool.tile([P, T], F32)
            yb_t = cpool.tile([P, T], F32)
            negx_t = cpool.tile([P, NT], F32)
            negy_t = cpool.tile([P, NT], F32)
            nslope_t = cpool.tile([P, HPC], F32)
            nc.sync.dma_start(out=xb_t[:], in_=xb[:])
            nc.sync.dma_start(out=yb_t[:], in_=yb[:])
            nc.sync.dma_start(out=negx_t[:], in_=negx[:])
            nc.sync.dma_start(out=negy_t[:], in_=negy[:])
            nc.sync.dma_start(out=nslope_t[:], in_=nslope[:])

            for t in range(NT):
                r0 = t * P
                dx2 = dxy_pool.tile([P, T], F32, tag="dx2")
                dy2 = dxy_pool.tile([P, T], F32, tag="dy2")
                # (xj - xi)^2 : Square(xb * 1 + (-xi)), bias is per-partition
                nc.scalar.activation(
                    dx2[:], xb_t[:], AF.Square, bias=negx_t[:, t : t + 1]
                )
                nc.scalar.activation(
                    dy2[:], yb_t[:], AF.Square, bias=negy_t[:, t : t + 1]
                )
                nc.vector.tensor_tensor(dx2[:], dx2[:], dy2[:], OP.add)
                dist = dist_pool.tile([P, T], F32)
                nc.scalar.activation(dist[:], dx2[:], AF.Sqrt)
                for h in range(HPC):
                    s = sin_pool.tile([P, T], F32)
                    nc.sync.dma_start(out=s[:], in_=scores[h, r0 : r0 + P, :])
                    o = sout_pool.tile([P, T], F32)
                    # out = (dist * -slope_h) + scores
                    nc.vector.scalar_tensor_tensor(
                        out=o[:],
                        in0=dist[:],
                        scalar=nslope_t[:, h : h + 1],
                        in1=s[:],
                        op0=OP.mult,
                        op1=OP.add,
                    )
                    # stores on the scalar engine's HWDGE ring so a store
                    # waiting on compute can't head-of-line-block loads on
                    # the sync ring
                    nc.scalar.dma_start(out=out[h, r0 : r0 + P, :], in_=o[:])
    nc.compile()
    return nc


def _shard_inputs(attn_scores, coords_xy, slopes):
    coords = coords_xy.astype(np.float32)
    slopes = np.asarray(slopes, dtype=np.float32)
    in_maps = []
    for c in range(NCORES):
        b = c // CPB
        h0 = (c % CPB) * HPC
        x = coords[b, :, 0]
        y = coords[b, :, 1]
        in_maps.append(
            {
                "scores": np.ascontiguousarray(attn_scores[b, h0 : h0 + HPC]),
                "xb": np.ascontiguousarray(np.broadcast_to(x[None, :], (P, T))),
                "yb": np.ascontiguousarray(np.broadcast_to(y[None, :], (P, T))),
                "negx": np.ascontiguousarray(-x.reshape(NT, P).T),
                "negy": np.ascontiguousarray(-y.reshape(NT, P).T),
                "nslope": np.ascontiguousarray(
                    np.broadcast_to(-slopes[h0 : h0 + HPC][None, :], (P, HPC))
                ),
            }
        )
    return in_maps


def _run(attn_scores, coords_xy, slopes, trace=False):
    attn_scores = np.asarray(attn_scores, dtype=np.float32)
    coords_xy = np.asarray(coords_xy)
    nc = _build_nc()
    in_maps = _shard_inputs(attn_scores, coords_xy, slopes)
    res = run_bass_kernel_spmd(nc, in_maps, core_ids=list(range(NCORES)), trace=trace)
    full = np.empty((B, H, T, T), dtype=np.float32)
    for c in range(NCORES):
        b = c // CPB
        h0 = (c % CPB) * HPC
        full[b, h0 : h0 + HPC] = res.results[c]["out"]
    return full, res


def kernel(attn_scores, coords_xy, slopes):
    full, _ = _run(attn_scores, coords_xy, slopes, trace=False)
    return full
